# revision 1
# baseline (speedup 1.0000x reference)
"""Trainium2 Bass kernel for nn_DecoderBlockBVL (B=2,V=8,L=256,C=768,H=12).

Sharding: 8 cores; core c handles batch b=c//4, query-slice g=c%4
(rows [g*64,(g+1)*64) of every view). Phase 1 (per-view self-attn) is
computed redundantly for the whole batch on each core (phase 2 needs
k/v for all tokens); phase-2 queries and the MLP cover only the core's
512 tokens. The host permutes each view's rows so the core's slice
sits at the front -> every core runs one identical SPMD program.

Layouts: residual stream token-major [tok, C]; matmul operands
feature-major (x^T) via PE transpose after each LN; weights
pre-transposed on the host to [C_in, F]. Matmuls run in float32r
(full PE rate at moving dim >= 256); attention probs/V and the fc2
operands in bf16.
"""

import numpy as np
import ml_dtypes

import concourse.bass as bass
import concourse.bacc as bacc
import concourse.mybir as mybir
import concourse.tile as tile
from concourse.bass_utils import run_bass_kernel_spmd
from concourse.masks import make_identity

dt = mybir.dt
F32 = dt.float32
F32R = dt.float32r
BF16 = dt.bfloat16
AF = mybir.ActivationFunctionType
ALU = mybir.AluOpType

B, V, L, C, H = 2, 8, 256, 768, 12
HD = C // H          # 64
S = V * L            # 2048
HID = 3072
NCORES = 8
G = 4                # cores per batch
QS = L // G          # 64 queries per view per core
MYQ = V * QS         # 512 tokens per core
SCALE = HD ** -0.5
CK = C // 128        # 6
SK = S // 128        # 16
HK = HID // 128      # 24
NHALF = ((0, 384), (384, 384))


def _kr(v):
    """allowed key prefix length for query view v (block-causal mask)"""
    return 512 if v < 2 else 256 * (v + 1)


def _build(ln_identity: bool, zero_bias: bool, sim_gelu: bool = False):
    nc = bacc.Bacc()

    xb = nc.declare_dram_parameter("xb", [S, C], F32, isOutput=False)
    wqkv = nc.declare_dram_parameter("wqkv_t", [C, 3 * C], BF16, isOutput=False)
    wproj = nc.declare_dram_parameter("wproj_t", [C, C], BF16, isOutput=False)
    wq = nc.declare_dram_parameter("wq_t", [C, C], BF16, isOutput=False)
    wk = nc.declare_dram_parameter("wk_t", [C, C], BF16, isOutput=False)
    wv = nc.declare_dram_parameter("wv_t", [C, C], BF16, isOutput=False)
    wcp = nc.declare_dram_parameter("wcproj_t", [C, C], BF16, isOutput=False)
    wf1 = nc.declare_dram_parameter("wfc1_t", [C, HID], BF16, isOutput=False)
    wf2 = nc.declare_dram_parameter("wfc2_t", [HID, C], BF16, isOutput=False)
    out = nc.declare_dram_parameter("out", [MYQ, C], F32, isOutput=True)

    lng = lnb = bias = f1b = None
    if not ln_identity:
        lng = nc.declare_dram_parameter("ln_g", [3, C], F32, isOutput=False)
        lnb = nc.declare_dram_parameter("ln_b", [3, C], F32, isOutput=False)
    if not zero_bias:
        bias = nc.declare_dram_parameter("bias3", [3, C], F32, isOutput=False)
        f1b = nc.declare_dram_parameter("fc1_b", [HID], F32, isOutput=False)

    x1d = nc.dram_tensor("x1d", [S, C], F32)  # phase-1 output spill

    with tile.TileContext(nc) as tc, \
         tc.tile_pool(name="consts", bufs=1) as consts:
        identb = consts.tile([128, 128], BF16)
        make_identity(nc, identb)
        eps = consts.tile([128, 1], F32)
        nc.vector.memset(eps, 1e-5)

        gbt = bbt = bias_bc = f1b_t = None
        if not ln_identity:
            gbt = consts.tile([128, 3, C], F32)
            bbt = consts.tile([128, 3, C], F32)
            for t, src in ((gbt, lng), (bbt, lnb)):
                bc = bass.AP(tensor=src.tensor, offset=src.offset,
                             ap=[[0, 128]] + list(src.ap))
                nc.gpsimd.dma_start(out=t[:], in_=bc)
        if not zero_bias:
            bias_bc = consts.tile([128, 3, C], F32)
            bc = bass.AP(tensor=bias.tensor, offset=bias.offset,
                         ap=[[0, 128]] + list(bias.ap))
            nc.gpsimd.dma_start(out=bias_bc[:], in_=bc)
            f1b_t = consts.tile([128, HK], F32)
            nc.gpsimd.dma_start(out=f1b_t[:], in_=f1b.rearrange("(a p) -> p a", p=128))

        def ln(pool, x_ap, h_ap, which):
            """layernorm over free dim C; x_ap/h_ap [128, C]"""
            st = pool.tile([128, 3, 6], F32, tag="ln_st")
            for sg in range(3):
                nc.vector.bn_stats(out=st[:, sg, :],
                                   in_=x_ap[:, sg * 256:(sg + 1) * 256])
            mv = pool.tile([128, 2], F32, tag="ln_mv")
            nc.vector.bn_aggr(out=mv[:], in_=st[:])
            nm = pool.tile([128, 2], F32, tag="ln_nm")  # [neg-mean, rstd]
            nc.vector.tensor_scalar_mul(nm[:, 0:1], mv[:, 0:1], -1.0)
            nc.scalar.activation(nm[:, 1:2], mv[:, 1:2], AF.Sqrt, bias=eps[:])
            nc.vector.reciprocal(nm[:, 1:2], nm[:, 1:2])
            nc.vector.tensor_scalar(h_ap, x_ap, nm[:, 0:1], nm[:, 1:2],
                                    ALU.add, ALU.mult)
            if not ln_identity:
                nc.vector.tensor_mul(h_ap, h_ap, gbt[:, which, :])
                nc.vector.tensor_add(h_ap, h_ap, bbt[:, which, :])

        def transpose_cols(psp, src, dst, j, n):
            """n [128,128] bf16 blocks src(mc) -> dst[:, j, :] ([128, n*128])"""
            ps = psp.tile([128, n * 128], BF16, tag="scb")
            for mc in range(n):
                nc.tensor.matmul(ps[:, mc * 128:(mc + 1) * 128], src(mc),
                                 identb[:], is_transpose=True)
            nc.any.tensor_copy(dst[:, j, :], ps[:])

        # =================== phase 1: per-view self-attention ===================
        with tc.tile_pool(name="p1w", bufs=1) as p1w, \
             tc.tile_pool(name="p1b", bufs=2) as p1b, \
             tc.tile_pool(name="p1s", bufs=3) as p1s, \
             tc.tile_pool(name="ps_sc", bufs=2, space="PSUM") as ps_sc, \
             tc.tile_pool(name="ps_mb", bufs=2, space="PSUM") as ps_mb, \
             tc.tile_pool(name="ps_o", bufs=2, space="PSUM") as ps_o:

            wqkv_s = p1w.tile([128, CK, 3 * C], BF16)
            wproj_s = p1w.tile([128, CK, C], BF16)
            for kc in range(CK):
                nc.sync.dma_start(out=wqkv_s[:, kc, :], in_=wqkv[kc * 128:(kc + 1) * 128, :])
                nc.sync.dma_start(out=wproj_s[:, kc, :], in_=wproj[kc * 128:(kc + 1) * 128, :])

            for v in range(V):
                xv = p1b.tile([128, 2, C], F32, tag="xv")
                for mc in range(2):
                    nc.sync.dma_start(out=xv[:, mc, :],
                                      in_=xb[v * L + mc * 128: v * L + (mc + 1) * 128, :])
                h1 = p1b.tile([128, 2, C], BF16, tag="h1")
                for mc in range(2):
                    ln(p1s, xv[:, mc, :], h1[:, mc, :], 0)
                h1T = p1b.tile([128, CK, 256], BF16, tag="h1T")
                for j in range(CK):
                    transpose_cols(ps_sc, lambda mc: h1[:, mc, j * 128:(j + 1) * 128],
                                   h1T, j, 2)

                # q^T,k^T feature-major [1536, 256]
                qkT = p1b.tile([128, 12, 256], BF16, tag="qkT")
                for mo in range(12):
                    ps = ps_sc.tile([128, 256], F32, tag="sc")
                    for kc in range(CK):
                        nc.tensor.matmul(ps[:], wqkv_s[:, kc, mo * 128:(mo + 1) * 128],
                                         h1T[:, kc, :], start=kc == 0, stop=kc == CK - 1)
                    nc.any.tensor_copy(qkT[:, mo, :], ps[:])
                # v token-major bf16 [256, 768]
                v1 = p1b.tile([128, 2, C], BF16, tag="v1")
                for mt in range(2):
                    pss = [ps_mb.tile([128, 384], F32, tag="mb", name=f"mbh{i}") for i in range(2)]
                    for kc in range(CK):
                        for i, (no, nn_) in enumerate(NHALF):
                            nc.tensor.matmul(pss[i][:],
                                             h1T[:, kc, mt * 128:(mt + 1) * 128],
                                             wqkv_s[:, kc, 2 * C + no:2 * C + no + nn_],
                                             start=kc == 0, stop=kc == CK - 1)
                    for i, (no, nn_) in enumerate(NHALF):
                        nc.any.tensor_copy(v1[:, mt, no:no + nn_], pss[i][:])

                o1T = p1b.tile([128, CK, 256], BF16, tag="o1T")
                for hp in range(6):
                    ops = ps_o.tile([128, 256], F32, tag="o")
                    for hh in range(2):
                        h = hp * 2 + hh
                        qh = qkT[hh * 64:(hh + 1) * 64, hp, :]       # [64, 256]
                        kh = qkT[hh * 64:(hh + 1) * 64, 6 + hp, :]   # [64, 256]
                        sps = ps_sc.tile([128, 2, 256], F32, tag="sc")
                        for mc in range(2):
                            nc.tensor.matmul(sps[:, mc, :], qh[:, mc * 128:(mc + 1) * 128],
                                             kh, start=True, stop=True)
                        probs = p1s.tile([128, 2, 256], BF16, tag="probs")
                        sums = p1s.tile([128, 2], F32, tag="sums")
                        for mc in range(2):
                            nc.scalar.activation(probs[:, mc, :], sps[:, mc, :], AF.Exp,
                                                 scale=SCALE, accum_out=sums[:, mc:mc + 1])
                        nc.vector.reciprocal(sums[:], sums[:])
                        for mc in range(2):
                            nc.vector.tensor_scalar_mul(probs[:, mc, :], probs[:, mc, :],
                                                        sums[:, mc:mc + 1])
                        pTps = ps_sc.tile([128, 2, 256], BF16, tag="scb")
                        for kb in range(2):
                            for mc in range(2):
                                nc.tensor.matmul(pTps[:, kb, mc * 128:(mc + 1) * 128],
                                                 probs[:, mc, kb * 128:(kb + 1) * 128],
                                                 identb[:], is_transpose=True)
                        pT = p1s.tile([128, 2, 256], BF16, tag="pT")
                        for kb in range(2):
                            nc.any.tensor_copy(pT[:, kb, :], pTps[:, kb, :])
                        for kb in range(2):
                            nc.tensor.matmul(ops[hh * 64:(hh + 1) * 64, :],
                                             v1[:, kb, h * 64:(h + 1) * 64], pT[:, kb, :],
                                             start=kb == 0, stop=kb == 1)
                    nc.any.tensor_copy(o1T[:, hp, :], ops[:])

                # proj + residual -> x1 (token-major), spill to DRAM
                for mt in range(2):
                    pss = [ps_mb.tile([128, 384], F32, tag="mb", name=f"mbh{i}") for i in range(2)]
                    for kc in range(CK):
                        for i, (no, nn_) in enumerate(NHALF):
                            nc.tensor.matmul(pss[i][:],
                                             o1T[:, kc, mt * 128:(mt + 1) * 128],
                                             wproj_s[:, kc, no:no + nn_],
                                             start=kc == 0, stop=kc == CK - 1)
                    x1v = p1b.tile([128, C], F32, tag="x1v")
                    for i, (no, nn_) in enumerate(NHALF):
                        nc.vector.tensor_add(x1v[:, no:no + nn_], pss[i][:],
                                             xv[:, mt, no:no + nn_])
                    if not zero_bias:
                        nc.vector.tensor_add(x1v[:], x1v[:], bias_bc[:, 0, :])
                    nc.sync.dma_start(out=x1d[v * L + mt * 128: v * L + (mt + 1) * 128, :],
                                      in_=x1v[:])

        # =================== phase 2 + 3 ===================
        with tc.tile_pool(name="p23", bufs=1) as p23:
            x2 = p23.tile([128, 4, C], F32)

            with tc.tile_pool(name="p2p", bufs=1) as p2p:
                k2T = p2p.tile([128, CK, S], BF16)
                v2 = p2p.tile([128, SK, C], BF16)
                h2mT = p2p.tile([128, CK, MYQ], BF16)
                q2T = p2p.tile([128, CK, MYQ], BF16)
                o2T = p2p.tile([128, CK, MYQ], BF16)

                # --- 2a: ln2 + k/v projections, streamed per view ---
                with tc.tile_pool(name="p2aw", bufs=1) as p2aw, \
                     tc.tile_pool(name="p2ab", bufs=2) as p2ab, \
                     tc.tile_pool(name="p2as", bufs=3) as p2as, \
                     tc.tile_pool(name="ps2_sc", bufs=2, space="PSUM") as ps2_sc, \
                     tc.tile_pool(name="ps2_mb", bufs=3, space="PSUM") as ps2_mb:
                    wk_s = p2aw.tile([128, CK, C], BF16)
                    wv_s = p2aw.tile([128, CK, C], BF16)
                    for kc in range(CK):
                        nc.sync.dma_start(out=wk_s[:, kc, :], in_=wk[kc * 128:(kc + 1) * 128, :])
                        nc.sync.dma_start(out=wv_s[:, kc, :], in_=wv[kc * 128:(kc + 1) * 128, :])

                    for v in range(V):
                        x1v = p2ab.tile([128, 2, C], F32, tag="x1v")
                        for mc in range(2):
                            nc.sync.dma_start(
                                out=x1v[:, mc, :],
                                in_=x1d[v * L + mc * 128: v * L + (mc + 1) * 128, :])
                        h2 = p2ab.tile([128, 2, C], BF16, tag="h2")
                        for mc in range(2):
                            ln(p2as, x1v[:, mc, :], h2[:, mc, :], 1)
                        h2T = p2ab.tile([128, CK, 256], BF16, tag="h2T")
                        for j in range(CK):
                            transpose_cols(ps2_sc,
                                           lambda mc: h2[:, mc, j * 128:(j + 1) * 128],
                                           h2T, j, 2)
                        for kc in range(CK):
                            nc.any.tensor_copy(h2mT[:, kc, v * QS:(v + 1) * QS],
                                               h2T[:, kc, 0:QS])
                        for mo in range(CK):
                            ps = ps2_sc.tile([128, 256], F32, tag="sc")
                            for kc in range(CK):
                                nc.tensor.matmul(ps[:],
                                                 wk_s[:, kc, mo * 128:(mo + 1) * 128],
                                                 h2T[:, kc, :],
                                                 start=kc == 0, stop=kc == CK - 1)
                            nc.any.tensor_copy(k2T[:, mo, v * L:(v + 1) * L], ps[:])
                        for mt in range(2):
                            pss = [ps2_mb.tile([128, 384], F32, tag="mb", name=f"mbh{i}") for i in range(2)]
                            for kc in range(CK):
                                for i, (no, nn_) in enumerate(NHALF):
                                    nc.tensor.matmul(pss[i][:],
                                                     h2T[:, kc, mt * 128:(mt + 1) * 128],
                                                     wv_s[:, kc, no:no + nn_],
                                                     start=kc == 0, stop=kc == CK - 1)
                            for i, (no, nn_) in enumerate(NHALF):
                                nc.any.tensor_copy(v2[:, v * 2 + mt, no:no + nn_], pss[i][:])

                # --- q projection for my 512 tokens ---
                with tc.tile_pool(name="p2qw", bufs=1) as p2qw, \
                     tc.tile_pool(name="ps2q", bufs=2, space="PSUM") as ps2q:
                    wq_s = p2qw.tile([128, CK, C], BF16)
                    for kc in range(CK):
                        nc.sync.dma_start(out=wq_s[:, kc, :], in_=wq[kc * 128:(kc + 1) * 128, :])
                    for mo in range(CK):
                        ps = ps2q.tile([128, MYQ], F32)
                        for kc in range(CK):
                            nc.tensor.matmul(ps[:], wq_s[:, kc, mo * 128:(mo + 1) * 128],
                                             h2mT[:, kc, :],
                                             start=kc == 0, stop=kc == CK - 1)
                        nc.any.tensor_copy(q2T[:, mo, :], ps[:])

                # --- 2b: block-causal attention over key prefixes ---
                with tc.tile_pool(name="p2bs", bufs=3) as p2bs, \
                     tc.tile_pool(name="ps2b_sc", bufs=3, space="PSUM") as ps2b_sc, \
                     tc.tile_pool(name="ps2b_o", bufs=2, space="PSUM") as ps2b_o:
                    for hp in range(6):
                        ops = ps2b_o.tile([128, MYQ], F32, tag="o")
                        for hh in range(2):
                            h = hp * 2 + hh
                            for v in range(V):
                                kr = _kr(v)
                                nk = (kr + 511) // 512
                                nkb = kr // 128
                                qh = q2T[hh * 64:(hh + 1) * 64, hp, v * QS:(v + 1) * QS]
                                probs = p2bs.tile([64, S], BF16, tag="probs2")
                                sums = p2bs.tile([64, 4], F32, tag="sums2")
                                for ck in range(nk):
                                    kw = min(512, kr - ck * 512)
                                    sps = ps2b_sc.tile([64, 512], F32, tag="sc")
                                    nc.tensor.matmul(sps[:, :kw], qh,
                                                     k2T[hh * 64:(hh + 1) * 64, hp,
                                                         ck * 512:ck * 512 + kw],
                                                     start=True, stop=True)
                                    nc.scalar.activation(probs[:, ck * 512:ck * 512 + kw],
                                                         sps[:, :kw], AF.Exp, scale=SCALE,
                                                         accum_out=sums[:, ck:ck + 1])
                                rtot = p2bs.tile([64, 1], F32, tag="rtot")
                                nc.vector.reduce_sum(out=rtot[:], in_=sums[:, 0:nk],
                                                     axis=mybir.AxisListType.X)
                                nc.vector.reciprocal(rtot[:], rtot[:])
                                nc.vector.tensor_scalar_mul(probs[:, :kr], probs[:, :kr],
                                                            rtot[:])
                                pT = p2bs.tile([128, SK, QS], BF16, tag="pT2")
                                for g4 in range((nkb + 3) // 4):
                                    nb = min(4, nkb - g4 * 4)
                                    pTps = ps2b_sc.tile([128, 4, QS], BF16, tag="scb")
                                    for i in range(nb):
                                        kb = g4 * 4 + i
                                        nc.tensor.matmul(pTps[:, i, :],
                                                         probs[:, kb * 128:(kb + 1) * 128],
                                                         identb[0:64, 0:QS],
                                                         is_transpose=True)
                                    nc.any.tensor_copy(pT[:, g4 * 4:g4 * 4 + nb, :],
                                                       pTps[:, 0:nb, :])
                                for kb in range(nkb):
                                    nc.tensor.matmul(
                                        ops[hh * 64:(hh + 1) * 64, v * QS:(v + 1) * QS],
                                        v2[:, kb, h * 64:(h + 1) * 64], pT[:, kb, :],
                                        start=kb == 0, stop=kb == nkb - 1)
                        nc.any.tensor_copy(o2T[:, hp, :], ops[:])

                # --- 2c: cproj + residual ---
                with tc.tile_pool(name="p2cw", bufs=1) as p2cw, \
                     tc.tile_pool(name="p2cs", bufs=2) as p2cs, \
                     tc.tile_pool(name="ps2c", bufs=3, space="PSUM") as ps2c:
                    wcp_s = p2cw.tile([128, CK, C], BF16)
                    for kc in range(CK):
                        nc.sync.dma_start(out=wcp_s[:, kc, :],
                                          in_=wcp[kc * 128:(kc + 1) * 128, :])
                    x1m = p2cw.tile([128, 4, C], F32)
                    for v in range(V):
                        nc.sync.dma_start(out=x1m[(v % 2) * 64:(v % 2) * 64 + 64, v // 2, :],
                                          in_=x1d[v * L: v * L + QS, :])
                    for mt in range(4):
                        pss = [ps2c.tile([128, 384], F32, tag="mb", name=f"mbh{i}") for i in range(2)]
                        for kc in range(CK):
                            for i, (no, nn_) in enumerate(NHALF):
                                nc.tensor.matmul(pss[i][:],
                                                 o2T[:, kc, mt * 128:(mt + 1) * 128],
                                                 wcp_s[:, kc, no:no + nn_],
                                                 start=kc == 0, stop=kc == CK - 1)
                        for i, (no, nn_) in enumerate(NHALF):
                            nc.vector.tensor_add(x2[:, mt, no:no + nn_], pss[i][:],
                                                 x1m[:, mt, no:no + nn_])
                        if not zero_bias:
                            nc.vector.tensor_add(x2[:, mt, :], x2[:, mt, :],
                                                 bias_bc[:, 1, :])

            # =================== phase 3: MLP ===================
            with tc.tile_pool(name="p3w", bufs=1) as p3w, \
                 tc.tile_pool(name="p3one", bufs=1) as p3one, \
                 tc.tile_pool(name="p3s", bufs=3) as p3s, \
                 tc.tile_pool(name="ps3_sc", bufs=2, space="PSUM") as ps3_sc, \
                 tc.tile_pool(name="ps3_mb", bufs=3, space="PSUM") as ps3_mb:
                wf1_s = p3w.tile([128, CK, HID], BF16)
                for kc in range(CK):
                    nc.sync.dma_start(out=wf1_s[:, kc, :], in_=wf1[kc * 128:(kc + 1) * 128, :])
                wf2_s = p3w.tile([128, HK, C], BF16)
                for kc in range(HK):
                    nc.sync.dma_start(out=wf2_s[:, kc, :], in_=wf2[kc * 128:(kc + 1) * 128, :])

                h3 = p3one.tile([128, 4, C], BF16)
                for mt in range(4):
                    ln(p3s, x2[:, mt, :], h3[:, mt, :], 2)
                h3T = p3one.tile([128, CK, MYQ], BF16)
                for j in range(CK):
                    transpose_cols(ps3_sc, lambda mc: h3[:, mc, j * 128:(j + 1) * 128],
                                   h3T, j, 4)
                g1T = p3one.tile([128, HK, MYQ], BF16)
                for mo in range(HK):
                    ps = ps3_sc.tile([128, MYQ], F32, tag="sc")
                    for kc in range(CK):
                        nc.tensor.matmul(ps[:], wf1_s[:, kc, mo * 128:(mo + 1) * 128],
                                         h3T[:, kc, :], start=kc == 0, stop=kc == CK - 1)
                    if sim_gelu:
                        # tanh-approx gelu from sim-supported ops (sim only)
                        xg = p3s.tile([128, MYQ], F32, tag="xg")
                        if zero_bias:
                            nc.any.tensor_copy(xg[:], ps[:])
                        else:
                            nc.scalar.activation(xg[:], ps[:], AF.Identity,
                                                 bias=f1b_t[:, mo:mo + 1])
                        x2g = p3s.tile([128, MYQ], F32, tag="x2g")
                        nc.scalar.activation(x2g[:], xg[:], AF.Square)
                        nc.vector.tensor_scalar(x2g[:], x2g[:], 0.0356774081,
                                                0.7978845608, ALU.mult, ALU.add)
                        nc.vector.tensor_mul(x2g[:], x2g[:], xg[:])
                        nc.scalar.activation(x2g[:], x2g[:], AF.Tanh)
                        nc.vector.tensor_mul(x2g[:], x2g[:], xg[:])
                        nc.vector.tensor_add(x2g[:], x2g[:], xg[:])
                        nc.vector.tensor_scalar_mul(x2g[:], x2g[:], 0.5)
                        nc.any.tensor_copy(g1T[:, mo, :], x2g[:])
                    elif zero_bias:
                        nc.scalar.activation(g1T[:, mo, :], ps[:], AF.Gelu)
                    else:
                        nc.scalar.activation(g1T[:, mo, :], ps[:], AF.Gelu,
                                             bias=f1b_t[:, mo:mo + 1])
                for mt in range(4):
                    pss = [ps3_mb.tile([128, 384], F32, tag="mb", name=f"mbh{i}") for i in range(2)]
                    for kc in range(HK):
                        for i, (no, nn_) in enumerate(NHALF):
                            nc.tensor.matmul(pss[i][:],
                                             g1T[:, kc, mt * 128:(mt + 1) * 128],
                                             wf2_s[:, kc, no:no + nn_],
                                             start=kc == 0, stop=kc == HK - 1)
                    yo = p3s.tile([128, C], F32, tag="yo")
                    for i, (no, nn_) in enumerate(NHALF):
                        nc.vector.tensor_add(yo[:, no:no + nn_], pss[i][:],
                                             x2[:, mt, no:no + nn_])
                    if not zero_bias:
                        nc.vector.tensor_add(yo[:], yo[:], bias_bc[:, 2, :])
                    nc.sync.dma_start(out=out[mt * 128:(mt + 1) * 128, :], in_=yo[:])

    nc.finalize()
    return nc


_CACHE = {}


def _get_nc(ln_identity, zero_bias, sim_gelu=False):
    key = (ln_identity, zero_bias, sim_gelu)
    if key not in _CACHE:
        _CACHE[key] = _build(ln_identity, zero_bias, sim_gelu)
    return _CACHE[key]


def _prep_inputs(inputs):
    x = np.asarray(inputs["x"], np.float32)          # [B, V, L, C]
    ln_identity = all(np.all(np.asarray(inputs[f"ln{i}_g"]) == 1.0)
                      and np.all(np.asarray(inputs[f"ln{i}_b"]) == 0.0)
                      for i in (1, 2, 3))
    zero_bias = all(np.all(np.asarray(inputs[k]) == 0.0)
                    for k in ("attn_proj_b", "cproj_b", "fc1_b", "fc2_b"))

    tr = lambda k: np.ascontiguousarray(
        np.asarray(inputs[k], np.float32).T).astype(ml_dtypes.bfloat16)
    wqkv_t, wproj_t = tr("qkv_w"), tr("attn_proj_w")
    wq_t, wk_t, wv_t, wcp_t = tr("q_w"), tr("k_w"), tr("v_w"), tr("cproj_w")
    wf1_t = tr("fc1_w")
    wf2_t = tr("fc2_w")

    in_maps = []
    for c in range(NCORES):
        b, g = divmod(c, G)
        xbp = np.empty((S, C), np.float32)
        for v in range(V):
            xv = x[b, v]
            xbp[v * L: v * L + QS] = xv[g * QS:(g + 1) * QS]
            xbp[v * L + QS: v * L + QS + g * QS] = xv[0: g * QS]
            xbp[v * L + QS + g * QS: (v + 1) * L] = xv[(g + 1) * QS:]
        m = {"xb": xbp, "wqkv_t": wqkv_t, "wproj_t": wproj_t, "wq_t": wq_t,
             "wk_t": wk_t, "wv_t": wv_t, "wcproj_t": wcp_t, "wfc1_t": wf1_t,
             "wfc2_t": wf2_t}
        if not ln_identity:
            m["ln_g"] = np.stack([np.asarray(inputs[f"ln{i}_g"], np.float32)
                                  for i in (1, 2, 3)])
            m["ln_b"] = np.stack([np.asarray(inputs[f"ln{i}_b"], np.float32)
                                  for i in (1, 2, 3)])
        if not zero_bias:
            m["bias3"] = np.stack([np.asarray(inputs["attn_proj_b"], np.float32),
                                   np.asarray(inputs["cproj_b"], np.float32),
                                   np.asarray(inputs["fc2_b"], np.float32)])
            m["fc1_b"] = np.asarray(inputs["fc1_b"], np.float32)
        in_maps.append(m)
    return in_maps, ln_identity, zero_bias


def _assemble(results):
    out = np.empty((B, V, L, C), np.float32)
    for c in range(NCORES):
        b, g = divmod(c, G)
        oc = np.asarray(results[c]["out"])
        for v in range(V):
            out[b, v, g * QS:(g + 1) * QS] = oc[v * QS:(v + 1) * QS]
    return out


def kernel(**inputs):
    in_maps, ln_identity, zero_bias = _prep_inputs(inputs)
    nc = _get_nc(ln_identity, zero_bias)
    res = run_bass_kernel_spmd(nc, in_maps, core_ids=list(range(NCORES)))
    return _assemble(res.results)



# revision 17
# speedup vs baseline: 154.8257x; 154.8257x over previous
"""Trainium2 Bass kernel for nn_DecoderBlockBVL (B=2,V=8,L=256,C=768,H=12).

Sharding: 8 cores; core c handles batch b=c//4 and owns view pair
(vA, vB) = (g, 7-g) with g=c%4 (512 tokens). Phase 1 (per-view
self-attn) and the phase-2 k/v/q projections run only on owned views;
k/v for all 8 views are exchanged in fp8 via two AllGathers over the
4-core batch group (zone 0 = views 0-3 = every rank's vA, zone 1 =
views 4-7 = every rank's vB), each overlapped with compute. The
program is identical on every core (SPMD): phase-2b runs a uniform 16
key-blocks with both owned query views merged into one 512-column
moving operand, and the block-causal mask is applied through a
per-core data input `mbias` — an additive bias of -30000 on the exp
activation for out-of-prefix key chunks, which zeroes those
probabilities exactly.

Attention is transpose-free: scores are computed keys-major (K-block
stationary, Q moving), exp'd in place (psum -> sbuf bf16), then used
directly as the moving operand of AV (V-slice stationary) and of a
ones-matmul that produces the softmax denominators replicated across
output partitions; normalization is one reciprocal + one tensor_mul
per (head-pair) on the AV output. Head pairs pack the PE array: head
2i uses contract rows / output cols 0-63, head 2i+1 uses 64-127.
The head-pair loop is software-pipelined: scores/exp of pair hp are
interleaved with AV/sums of pair hp-1 so scalar-exp latency never
idles the PE.
"""

import numpy as np
import ml_dtypes
from contextlib import ExitStack

import concourse.bass as bass
import concourse.bacc as bacc
import concourse.mybir as mybir
import concourse.tile as tile
from concourse.bass_utils import run_bass_kernel_spmd
from concourse.masks import make_identity

dt = mybir.dt
F32 = dt.float32
BF16 = dt.bfloat16
FP8 = dt.float8e4
AF = mybir.ActivationFunctionType
ALU = mybir.AluOpType

B, V, L, C, H = 2, 8, 256, 768, 12
HD = C // H          # 64
S = V * L            # 2048
HID = 3072
NCORES = 8
G = 4                # cores per batch
MYQ = 2 * L          # 512 tokens per core (2 views)
SCALE = HD ** -0.5
CK = C // 128        # 6
HK = HID // 128      # 24
NHALF = ((0, 384), (384, 384))
NKB = 16             # uniform key-block count in phase 2b
MASKB = -30000.0     # exp bias for masked key chunks -> exp == 0.0

KZ = C * L           # one view's k zone [C, L] in agin, elems
VZ = L * C           # one view's v zone [L, C]
ZONE = KZ + VZ       # agin zone per view


def _kr(v):
    """allowed key prefix length for query view v (block-causal mask)"""
    return 512 if v < 2 else 256 * (v + 1)


def _build(ln_identity: bool, zero_bias: bool, sim_gelu: bool = False):
    nc = bacc.Bacc(num_devices=NCORES)

    xb = nc.declare_dram_parameter("xb", [MYQ, C], F32, isOutput=False)
    mbias = nc.declare_dram_parameter("mbias", [2, 16], F32, isOutput=False)
    wqkv = nc.declare_dram_parameter("wqkv_t", [C, 3 * C], BF16, isOutput=False)
    wproj = nc.declare_dram_parameter("wproj_t", [C, C], BF16, isOutput=False)
    wq = nc.declare_dram_parameter("wq_t", [C, C], BF16, isOutput=False)
    wk = nc.declare_dram_parameter("wk_t", [C, C], BF16, isOutput=False)
    wv = nc.declare_dram_parameter("wv_t", [C, C], BF16, isOutput=False)
    wcp = nc.declare_dram_parameter("wcproj_t", [C, C], BF16, isOutput=False)
    wf1 = nc.declare_dram_parameter("wfc1_t", [C, HID], BF16, isOutput=False)
    wf2 = nc.declare_dram_parameter("wfc2_t", [HID, C], BF16, isOutput=False)
    out = nc.declare_dram_parameter("out", [MYQ, C], F32, isOutput=True)

    lng = lnb = bias = f1b = None
    if not ln_identity:
        lng = nc.declare_dram_parameter("ln_g", [3, C], F32, isOutput=False)
        lnb = nc.declare_dram_parameter("ln_b", [3, C], F32, isOutput=False)
    if not zero_bias:
        bias = nc.declare_dram_parameter("bias3", [3, C], F32, isOutput=False)
        f1b = nc.declare_dram_parameter("fc1_b", [HID], F32, isOutput=False)

    rg = [[0, 1, 2, 3], [4, 5, 6, 7]]

    with tile.TileContext(nc) as tc, \
         tc.tile_pool(name="consts", bufs=1) as consts, \
         tc.tile_pool(name="dram", bufs=1, space="DRAM") as dram:
        identb = consts.tile([128, 128], BF16)
        make_identity(nc, identb)
        eps = consts.tile([128, 1], F32)
        nc.vector.memset(eps, 1e-5)
        ones64 = consts.tile([128, 64], BF16)
        nc.vector.memset(ones64, 1.0)

        def bcast_ap(handle):
            a = handle.ap()
            return bass.AP(tensor=a.tensor, offset=a.offset,
                           ap=[[0, 128]] + list(a.ap))

        mb_t = consts.tile([128, 2, 16], F32)
        nc.gpsimd.dma_start(out=mb_t[:], in_=bcast_ap(mbias))

        gbt = bbt = bias_bc = f1b_t = None
        if not ln_identity:
            gbt = consts.tile([128, 3, C], F32)
            bbt = consts.tile([128, 3, C], F32)
            for t, src in ((gbt, lng), (bbt, lnb)):
                nc.gpsimd.dma_start(out=t[:], in_=bcast_ap(src))
        if not zero_bias:
            bias_bc = consts.tile([128, 3, C], F32)
            nc.gpsimd.dma_start(out=bias_bc[:], in_=bcast_ap(bias))
            f1b_t = consts.tile([128, HK], F32)
            nc.gpsimd.dma_start(out=f1b_t[:], in_=f1b.rearrange("(a p) -> p a", p=128))

        agin = dram.tile([2 * ZONE], FP8)
        agout = [dram.tile([G * ZONE], FP8, name=f"agout{z}")
                 for z in range(2)]

        def ln(pool, x_ap, h_ap, which):
            """layernorm over free dim C; x_ap/h_ap [128, C]"""
            st = pool.tile([128, 3, 6], F32, tag="ln_st")
            for sg in range(3):
                nc.vector.bn_stats(out=st[:, sg, :],
                                   in_=x_ap[:, sg * 256:(sg + 1) * 256])
            mv = pool.tile([128, 2], F32, tag="ln_mv")
            nc.vector.bn_aggr(out=mv[:], in_=st[:])
            nm = pool.tile([128, 2], F32, tag="ln_nm")  # [neg-mean, rstd]
            nc.vector.tensor_scalar_mul(nm[:, 0:1], mv[:, 0:1], -1.0)
            nc.scalar.activation(nm[:, 1:2], mv[:, 1:2], AF.Sqrt, bias=eps[:])
            nc.vector.reciprocal(nm[:, 1:2], nm[:, 1:2])
            nc.vector.tensor_scalar(h_ap, x_ap, nm[:, 0:1], nm[:, 1:2],
                                    ALU.add, ALU.mult)
            if not ln_identity:
                nc.vector.tensor_mul(h_ap, h_ap, gbt[:, which, :])
                nc.vector.tensor_add(h_ap, h_ap, bbt[:, which, :])

        def transpose_cols(psp, src, dst, j, n):
            """n [128,128] bf16 blocks src(mc) -> dst[:, j, :] ([128, n*128])"""
            ps = psp.tile([128, n * 128], BF16, tag="scb")
            for mc in range(n):
                nc.tensor.matmul(ps[:, mc * 128:(mc + 1) * 128], src(mc),
                                 identb[:], is_transpose=True)
            nc.any.tensor_copy(dst[:, j, :], ps[:])

        # ============ long-lived activation tiles ============
        with tc.tile_pool(name="res", bufs=1) as res, \
             tc.tile_pool(name="p2q", bufs=1) as p2q:
            x1own = res.tile([128, 4, C], F32)
            x2 = res.tile([128, 4, C], F32)
            q2T = p2q.tile([128, CK, MYQ], BF16)

            # ============ phase 1 + 2a per owned view ============
            with tc.tile_pool(name="p1w", bufs=1) as p1w, \
                 tc.tile_pool(name="p1b", bufs=2) as p1b, \
                 tc.tile_pool(name="p1s", bufs=3) as p1s, \
                 tc.tile_pool(name="ps_sc", bufs=1, space="PSUM") as ps_sc, \
                 tc.tile_pool(name="ps_mb", bufs=1, space="PSUM") as ps_mb, \
                 tc.tile_pool(name="ps_s", bufs=1, space="PSUM") as ps_s, \
                 tc.tile_pool(name="ps_o", bufs=1, space="PSUM") as ps_o:

                wqkv_s = p1w.tile([128, CK, 3 * C], BF16)
                wproj_s = p1w.tile([128, CK, C], BF16)
                wk_s = p1w.tile([128, CK, C], BF16)
                wv_s = p1w.tile([128, CK, C], BF16)
                wq_s = p1w.tile([128, CK, C], BF16)
                for kc in range(CK):
                    nc.sync.dma_start(out=wqkv_s[:, kc, :],
                                      in_=wqkv[kc * 128:(kc + 1) * 128, :])
                for wt, ws in ((wproj, wproj_s), (wk, wk_s), (wv, wv_s),
                               (wq, wq_s)):
                    for kc in range(CK):
                        nc.sync.dma_start(out=ws[:, kc, :],
                                          in_=wt[kc * 128:(kc + 1) * 128, :])

                for vi in range(2):
                    # ---- phase 1: self-attention within owned view vi ----
                    xv = p1b.tile([128, 2, C], F32, tag="xv")
                    for mc in range(2):
                        nc.scalar.dma_start(
                            out=xv[:, mc, :],
                            in_=xb[vi * L + mc * 128: vi * L + (mc + 1) * 128, :])
                    h1 = p1b.tile([128, 2, C], BF16, tag="h1")
                    for mc in range(2):
                        ln(p1s, xv[:, mc, :], h1[:, mc, :], 0)
                    h1T = p1b.tile([128, CK, 256], BF16, tag="h1T")
                    for j in range(CK):
                        transpose_cols(ps_sc,
                                       lambda mc: h1[:, mc, j * 128:(j + 1) * 128],
                                       h1T, j, 2)
                    # q,k feature-major [1536, 256]
                    qkT = p1b.tile([128, 12, 256], BF16, tag="qkT")
                    for mo in range(12):
                        ps = ps_sc.tile([128, 256], F32, tag="sc")
                        for kc in range(CK):
                            nc.tensor.matmul(ps[:],
                                             wqkv_s[:, kc, mo * 128:(mo + 1) * 128],
                                             h1T[:, kc, :],
                                             start=kc == 0, stop=kc == CK - 1)
                        nc.any.tensor_copy(qkT[:, mo, :], ps[:])
                    # v token-major bf16 [256, 768]
                    v1 = p1b.tile([128, 2, C], BF16, tag="v1")
                    for mt in range(2):
                        pss = [ps_mb.tile([128, 384], F32, tag="mb",
                                          name=f"mbh{i}") for i in range(2)]
                        for kc in range(CK):
                            for i, (no, nn_) in enumerate(NHALF):
                                nc.tensor.matmul(pss[i][:],
                                                 h1T[:, kc, mt * 128:(mt + 1) * 128],
                                                 wqkv_s[:, kc, 2 * C + no:2 * C + no + nn_],
                                                 start=kc == 0, stop=kc == CK - 1)
                        for i, (no, nn_) in enumerate(NHALF):
                            nc.any.tensor_copy(v1[:, mt, no:no + nn_], pss[i][:])

                    # transpose-free attention; head-pair loop software-
                    # pipelined: scores/exp(hp) interleaved with AV(hp-1)
                    o1T = p1b.tile([128, CK, 256], BF16, tag="o1T")
                    prev = None
                    for hp in range(7):
                        cur = None
                        if hp < 6:
                            sA = ps_s.tile([128, 2, 256], F32, tag="sA")
                            sB = ps_s.tile([128, 2, 256], F32, tag="sB")
                            for j in range(2):
                                nc.tensor.matmul(
                                    sA[:, j, :],
                                    qkT[0:64, 6 + hp, j * 128:(j + 1) * 128],
                                    qkT[0:64, hp, :], start=True, stop=True)
                                nc.tensor.matmul(
                                    sB[:, j, :],
                                    qkT[64:128, 6 + hp, j * 128:(j + 1) * 128],
                                    qkT[64:128, hp, :], start=True, stop=True)
                            eA = p1s.tile([128, 2, 256], BF16, tag="eA1")
                            eB = p1s.tile([128, 2, 256], BF16, tag="eB1")
                            nc.scalar.activation(eA[:], sA[:], AF.Exp, scale=SCALE)
                            nc.scalar.activation(eB[:], sB[:], AF.Exp, scale=SCALE)
                            cur = (hp, eA, eB)
                        if prev is not None:
                            php, peA, peB = prev
                            hA, hB = 2 * php, 2 * php + 1
                            o_ps = ps_o.tile([128, 512], F32, tag="o_ps")
                            s_ps = ps_o.tile([128, 512], F32, tag="s_ps")
                            for kb in range(2):
                                st, sp = (kb == 0), (kb == 1)
                                nc.tensor.matmul(o_ps[0:64, 0:256],
                                                 v1[:, kb, hA * 64:hA * 64 + 64],
                                                 peA[:, kb, :], start=st, stop=sp,
                                                 tile_position=(0, 0),
                                                 skip_group_check=True)
                                nc.tensor.matmul(o_ps[64:128, 0:256],
                                                 v1[:, kb, hB * 64:hB * 64 + 64],
                                                 peB[:, kb, :], start=st, stop=sp,
                                                 tile_position=(0, 64),
                                                 skip_group_check=True)
                                nc.tensor.matmul(s_ps[0:64, 0:256], ones64[:],
                                                 peA[:, kb, :], start=st, stop=sp,
                                                 tile_position=(0, 0),
                                                 skip_group_check=True)
                                nc.tensor.matmul(s_ps[64:128, 0:256], ones64[:],
                                                 peB[:, kb, :], start=st, stop=sp,
                                                 tile_position=(0, 64),
                                                 skip_group_check=True)
                            rinv = p1s.tile([128, 256], F32, tag="rinv1")
                            nc.vector.reciprocal(rinv[:], s_ps[:, 0:256])
                            nc.vector.tensor_mul(o1T[:, php, :], o_ps[:, 0:256],
                                                 rinv[:])
                        prev = cur

                    # proj + residual -> x1own (token-major, stays in SBUF)
                    for mt in range(2):
                        pss = [ps_mb.tile([128, 384], F32, tag="mb",
                                          name=f"mbh{i}") for i in range(2)]
                        for kc in range(CK):
                            for i, (no, nn_) in enumerate(NHALF):
                                nc.tensor.matmul(pss[i][:],
                                                 o1T[:, kc, mt * 128:(mt + 1) * 128],
                                                 wproj_s[:, kc, no:no + nn_],
                                                 start=kc == 0, stop=kc == CK - 1)
                        for i, (no, nn_) in enumerate(NHALF):
                            nc.vector.tensor_add(x1own[:, vi * 2 + mt, no:no + nn_],
                                                 pss[i][:], xv[:, mt, no:no + nn_])
                        if not zero_bias:
                            nc.vector.tensor_add(x1own[:, vi * 2 + mt, :],
                                                 x1own[:, vi * 2 + mt, :],
                                                 bias_bc[:, 0, :])

                    # ---- phase 2a: ln2 + k/v/q projections for this view ----
                    h2 = p1b.tile([128, 2, C], BF16, tag="h2")
                    for mc in range(2):
                        ln(p1s, x1own[:, vi * 2 + mc, :], h2[:, mc, :], 1)
                    h2T = p1b.tile([128, CK, 256], BF16, tag="h2T")
                    for j in range(CK):
                        transpose_cols(ps_sc,
                                       lambda mc: h2[:, mc, j * 128:(j + 1) * 128],
                                       h2T, j, 2)
                    kown = p1b.tile([128, CK, 256], FP8, tag="kown")
                    for mo in range(CK):
                        ps = ps_sc.tile([128, 256], F32, tag="sc")
                        for kc in range(CK):
                            nc.tensor.matmul(ps[:],
                                             wk_s[:, kc, mo * 128:(mo + 1) * 128],
                                             h2T[:, kc, :],
                                             start=kc == 0, stop=kc == CK - 1)
                        nc.any.tensor_copy(kown[:, mo, :], ps[:])
                    vown = p1b.tile([128, 2, C], FP8, tag="vown")
                    for mt in range(2):
                        pss = [ps_mb.tile([128, 384], F32, tag="mb",
                                          name=f"mbh{i}") for i in range(2)]
                        for kc in range(CK):
                            for i, (no, nn_) in enumerate(NHALF):
                                nc.tensor.matmul(pss[i][:],
                                                 h2T[:, kc, mt * 128:(mt + 1) * 128],
                                                 wv_s[:, kc, no:no + nn_],
                                                 start=kc == 0, stop=kc == CK - 1)
                        for i, (no, nn_) in enumerate(NHALF):
                            nc.any.tensor_copy(vown[:, mt, no:no + nn_], pss[i][:])

                    # spill own k/v to agin zone vi, AllGather with the other
                    # ranks' same-zone views (zone 0: views 0-3, zone 1: 4-7)
                    zo = vi * ZONE
                    nc.sync.dma_start(
                        out=bass.AP(tensor=agin.tensor, offset=agin.offset + zo,
                                    ap=[[L, 128], [128 * L, CK], [1, L]]),
                        in_=kown[:])
                    nc.sync.dma_start(
                        out=bass.AP(tensor=agin.tensor, offset=agin.offset + zo + KZ,
                                    ap=[[C, 128], [128 * C, 2], [1, C]]),
                        in_=vown[:])
                    nc.gpsimd.collective_compute(
                        "AllGather", ALU.bypass, replica_groups=rg,
                        ins=[agin[zo:zo + ZONE].opt()],
                        outs=[agout[vi][:].opt()])

                    # q projection for this view (overlaps the collective)
                    for mo in range(CK):
                        ps = ps_sc.tile([128, 256], F32, tag="sc")
                        for kc in range(CK):
                            nc.tensor.matmul(ps[:],
                                             wq_s[:, kc, mo * 128:(mo + 1) * 128],
                                             h2T[:, kc, :],
                                             start=kc == 0, stop=kc == CK - 1)
                        nc.any.tensor_copy(q2T[:, mo, vi * 256:(vi + 1) * 256],
                                           ps[:])

            # ====== k/v tiles for phase 2b (fp8) ======
            stack2 = ExitStack()
            p2kv = stack2.enter_context(tc.tile_pool(name="p2kv", bufs=1))
            p3w = stack2.enter_context(tc.tile_pool(name="p3w", bufs=1))
            k2Tv = [p2kv.tile([128, CK, L], FP8, name=f"k2Tv{v}")
                    for v in range(V)]
            v2v = [p2kv.tile([128, 2, C], FP8, name=f"v2v{v}")
                   for v in range(V)]
            o2T = p2kv.tile([128, CK, MYQ], BF16, name="o2T")

            # readback: all ranks' k/v from the gathered zones
            for vi in range(2):
                for r in range(G):
                    v = r if vi == 0 else 7 - r
                    base = agout[vi].offset + r * ZONE
                    nc.sync.dma_start(
                        out=k2Tv[v][:],
                        in_=bass.AP(tensor=agout[vi].tensor, offset=base,
                                    ap=[[L, 128], [128 * L, CK], [1, L]]))
                    nc.sync.dma_start(
                        out=v2v[v][:],
                        in_=bass.AP(tensor=agout[vi].tensor, offset=base + KZ,
                                    ap=[[C, 128], [128 * C, 2], [1, C]]))

            # ====== phase 2b: block-causal cross-view attention ======
            # Both query views merged: columns 0:256 = vA queries, 256:512 =
            # vB queries. Key block kb covers canonical view kb//2; kb < 8
            # (views 0-3) feeds both query views, kb >= 8 only vB.
            with tc.tile_pool(name="p2bs", bufs=2) as p2bs, \
                 tc.tile_pool(name="ps2b_s", bufs=1, space="PSUM") as ps2b_s, \
                 tc.tile_pool(name="ps2b_o", bufs=2, space="PSUM") as ps2b_o:
                prev = None
                for hp in range(7):
                    cur = None
                    if hp < 6:
                        eA = p2bs.tile([128, NKB, MYQ], BF16, tag="eA")
                        eB = p2bs.tile([128, NKB, MYQ], BF16, tag="eB")
                        cur = (hp, eA, eB)
                    if prev is not None:
                        o_ps = ps2b_o.tile([128, MYQ], F32, tag="o_ps")
                        s_ps = ps2b_o.tile([128, MYQ], F32, tag="s_ps")
                    for kbp in range(NKB // 2):
                        if hp < 6:
                            sA = ps2b_s.tile([128, 2, MYQ], F32, tag="sA")
                            sB = ps2b_s.tile([128, 2, MYQ], F32, tag="sB")
                            for j in range(2):
                                kb = 2 * kbp + j
                                w, jj = kb // 2, kb % 2
                                co = 0 if kb < 8 else 256
                                nc.tensor.matmul(
                                    sA[:, j, co:MYQ],
                                    k2Tv[w][0:64, hp, jj * 128:(jj + 1) * 128],
                                    q2T[0:64, hp, co:MYQ],
                                    start=True, stop=True)
                                nc.tensor.matmul(
                                    sB[:, j, co:MYQ],
                                    k2Tv[w][64:128, hp, jj * 128:(jj + 1) * 128],
                                    q2T[64:128, hp, co:MYQ],
                                    start=True, stop=True)
                            for s_, e_ in ((sA, eA), (sB, eB)):
                                eo = e_[:, 2 * kbp:2 * kbp + 2, :]
                                if kbp < 2:
                                    # kb 0-3: no view is ever masked here
                                    nc.scalar.activation(eo[:, :, :], s_[:],
                                                         AF.Exp, scale=SCALE)
                                elif kbp < 4:
                                    # kb 4-7: vA may be masked; vB never
                                    nc.scalar.activation(
                                        eo[:, :, 0:256], s_[:, :, 0:256],
                                        AF.Exp, scale=SCALE,
                                        bias=mb_t[:, 0, kbp:kbp + 1])
                                    nc.scalar.activation(
                                        eo[:, :, 256:MYQ], s_[:, :, 256:MYQ],
                                        AF.Exp, scale=SCALE)
                                else:
                                    # kb 8-15: vB only, may be masked
                                    nc.scalar.activation(
                                        eo[:, :, 256:MYQ], s_[:, :, 256:MYQ],
                                        AF.Exp, scale=SCALE,
                                        bias=mb_t[:, 1, kbp:kbp + 1])
                        if prev is not None:
                            php, peA, peB = prev
                            hA, hB = 2 * php, 2 * php + 1
                            for j in range(2):
                                kb = 2 * kbp + j
                                w, jj = kb // 2, kb % 2
                                co = 0 if kb < 8 else 256
                                st, sp = (kb == 0), (kb == NKB - 1)
                                nc.tensor.matmul(o_ps[0:64, co:MYQ],
                                                 v2v[w][:, jj, hA * 64:hA * 64 + 64],
                                                 peA[:, kb, co:MYQ],
                                                 start=st, stop=sp,
                                                 tile_position=(0, 0),
                                                 skip_group_check=True)
                                nc.tensor.matmul(o_ps[64:128, co:MYQ],
                                                 v2v[w][:, jj, hB * 64:hB * 64 + 64],
                                                 peB[:, kb, co:MYQ],
                                                 start=st, stop=sp,
                                                 tile_position=(0, 64),
                                                 skip_group_check=True)
                                nc.tensor.matmul(s_ps[0:64, co:MYQ], ones64[:],
                                                 peA[:, kb, co:MYQ],
                                                 start=st, stop=sp,
                                                 tile_position=(0, 0),
                                                 skip_group_check=True)
                                nc.tensor.matmul(s_ps[64:128, co:MYQ], ones64[:],
                                                 peB[:, kb, co:MYQ],
                                                 start=st, stop=sp,
                                                 tile_position=(0, 64),
                                                 skip_group_check=True)
                    if prev is not None:
                        php = prev[0]
                        rinv = p2bs.tile([128, MYQ], F32, tag="rinv")
                        nc.vector.reciprocal(rinv[:], s_ps[:])
                        nc.vector.tensor_mul(o2T[:, php, :], o_ps[:], rinv[:])
                    prev = cur

            # ====== phase 2c: cproj + residual (+ fc1/fc2 prefetch) ======
            with tc.tile_pool(name="p2cw", bufs=1) as p2cw, \
                 tc.tile_pool(name="ps2c", bufs=3, space="PSUM") as ps2c:
                wcp_s = p2cw.tile([128, CK, C], BF16)
                for kc in range(CK):
                    nc.sync.dma_start(out=wcp_s[:, kc, :],
                                      in_=wcp[kc * 128:(kc + 1) * 128, :])
                wf1_s = p3w.tile([128, CK, HID], BF16)
                for kc in range(CK):
                    nc.gpsimd.dma_start(out=wf1_s[:, kc, :],
                                        in_=wf1[kc * 128:(kc + 1) * 128, :])
                wf2_s = p3w.tile([128, HK, C], BF16)
                for kc in range(HK):
                    nc.gpsimd.dma_start(out=wf2_s[:, kc, :],
                                        in_=wf2[kc * 128:(kc + 1) * 128, :])
                for mt in range(4):
                    pss = [ps2c.tile([128, 384], F32, tag="mb",
                                     name=f"mbh{i}") for i in range(2)]
                    for kc in range(CK):
                        for i, (no, nn_) in enumerate(NHALF):
                            nc.tensor.matmul(pss[i][:],
                                             o2T[:, kc, mt * 128:(mt + 1) * 128],
                                             wcp_s[:, kc, no:no + nn_],
                                             start=kc == 0, stop=kc == CK - 1)
                    for i, (no, nn_) in enumerate(NHALF):
                        nc.vector.tensor_add(x2[:, mt, no:no + nn_], pss[i][:],
                                             x1own[:, mt, no:no + nn_])
                    if not zero_bias:
                        nc.vector.tensor_add(x2[:, mt, :], x2[:, mt, :],
                                             bias_bc[:, 1, :])

            # ====== phase 3: MLP ======
            with tc.tile_pool(name="p3one", bufs=1) as p3one, \
                 tc.tile_pool(name="p3s", bufs=2) as p3s, \
                 tc.tile_pool(name="ps3_sc", bufs=2, space="PSUM") as ps3_sc, \
                 tc.tile_pool(name="ps3_mb", bufs=3, space="PSUM") as ps3_mb:
                h3 = p3one.tile([128, 4, C], BF16)
                for mt in range(4):
                    ln(p3s, x2[:, mt, :], h3[:, mt, :], 2)
                h3T = p3one.tile([128, CK, MYQ], BF16)
                for j in range(CK):
                    transpose_cols(ps3_sc,
                                   lambda mc: h3[:, mc, j * 128:(j + 1) * 128],
                                   h3T, j, 4)
                g1T = p3one.tile([128, HK, MYQ], BF16)
                for mo in range(HK):
                    ps = ps3_sc.tile([128, MYQ], F32, tag="sc")
                    for kc in range(CK):
                        nc.tensor.matmul(ps[:], wf1_s[:, kc, mo * 128:(mo + 1) * 128],
                                         h3T[:, kc, :], start=kc == 0, stop=kc == CK - 1)
                    if sim_gelu:
                        # tanh-approx gelu from sim-supported ops (sim only)
                        xg = p3s.tile([128, MYQ], F32, tag="xg")
                        if zero_bias:
                            nc.any.tensor_copy(xg[:], ps[:])
                        else:
                            nc.scalar.activation(xg[:], ps[:], AF.Identity,
                                                 bias=f1b_t[:, mo:mo + 1])
                        x2g = p3s.tile([128, MYQ], F32, tag="x2g")
                        nc.scalar.activation(x2g[:], xg[:], AF.Square)
                        nc.vector.tensor_scalar(x2g[:], x2g[:], 0.0356774081,
                                                0.7978845608, ALU.mult, ALU.add)
                        nc.vector.tensor_mul(x2g[:], x2g[:], xg[:])
                        nc.scalar.activation(x2g[:], x2g[:], AF.Tanh)
                        nc.vector.tensor_mul(x2g[:], x2g[:], xg[:])
                        nc.vector.tensor_add(x2g[:], x2g[:], xg[:])
                        nc.vector.tensor_scalar_mul(x2g[:], x2g[:], 0.5)
                        nc.any.tensor_copy(g1T[:, mo, :], x2g[:])
                    elif zero_bias:
                        nc.scalar.activation(g1T[:, mo, :], ps[:], AF.Gelu)
                    else:
                        nc.scalar.activation(g1T[:, mo, :], ps[:], AF.Gelu,
                                             bias=f1b_t[:, mo:mo + 1])
                for mt in range(4):
                    pss = [ps3_mb.tile([128, 384], F32, tag="mb",
                                       name=f"mbh{i}") for i in range(2)]
                    for kc in range(HK):
                        for i, (no, nn_) in enumerate(NHALF):
                            nc.tensor.matmul(pss[i][:],
                                             g1T[:, kc, mt * 128:(mt + 1) * 128],
                                             wf2_s[:, kc, no:no + nn_],
                                             start=kc == 0, stop=kc == HK - 1)
                    yo = p3s.tile([128, C], F32, tag="yo")
                    for i, (no, nn_) in enumerate(NHALF):
                        nc.vector.tensor_add(yo[:, no:no + nn_], pss[i][:],
                                             x2[:, mt, no:no + nn_])
                    if not zero_bias:
                        nc.vector.tensor_add(yo[:], yo[:], bias_bc[:, 2, :])
                    nc.sync.dma_start(out=out[mt * 128:(mt + 1) * 128, :], in_=yo[:])
            stack2.close()

    nc.finalize()
    return nc


_CACHE = {}


def _get_nc(ln_identity, zero_bias, sim_gelu=False):
    key = (ln_identity, zero_bias, sim_gelu)
    if key not in _CACHE:
        _CACHE[key] = _build(ln_identity, zero_bias, sim_gelu)
    return _CACHE[key]


def _prep_inputs(inputs):
    x = np.asarray(inputs["x"], np.float32)          # [B, V, L, C]
    ln_identity = all(np.all(np.asarray(inputs[f"ln{i}_g"]) == 1.0)
                      and np.all(np.asarray(inputs[f"ln{i}_b"]) == 0.0)
                      for i in (1, 2, 3))
    zero_bias = all(np.all(np.asarray(inputs[k]) == 0.0)
                    for k in ("attn_proj_b", "cproj_b", "fc1_b", "fc2_b"))

    tr = lambda k: np.ascontiguousarray(
        np.asarray(inputs[k], np.float32).T).astype(ml_dtypes.bfloat16)
    wqkv_t, wproj_t = tr("qkv_w"), tr("attn_proj_w")
    wq_t, wk_t, wv_t, wcp_t = tr("q_w"), tr("k_w"), tr("v_w"), tr("cproj_w")
    wf1_t = tr("fc1_w")
    wf2_t = tr("fc2_w")

    in_maps = []
    for c in range(NCORES):
        b, g = divmod(c, G)
        xbp = np.concatenate([x[b, g], x[b, 7 - g]], axis=0)  # [512, C]
        mb = np.zeros((2, 16), np.float32)
        for vi, v in enumerate((g, 7 - g)):
            mb[vi, _kr(v) // 256:] = MASKB
        m = {"xb": np.ascontiguousarray(xbp), "mbias": mb,
             "wqkv_t": wqkv_t, "wproj_t": wproj_t, "wq_t": wq_t,
             "wk_t": wk_t, "wv_t": wv_t, "wcproj_t": wcp_t,
             "wfc1_t": wf1_t, "wfc2_t": wf2_t}
        if not ln_identity:
            m["ln_g"] = np.stack([np.asarray(inputs[f"ln{i}_g"], np.float32)
                                  for i in (1, 2, 3)])
            m["ln_b"] = np.stack([np.asarray(inputs[f"ln{i}_b"], np.float32)
                                  for i in (1, 2, 3)])
        if not zero_bias:
            m["bias3"] = np.stack([np.asarray(inputs["attn_proj_b"], np.float32),
                                   np.asarray(inputs["cproj_b"], np.float32),
                                   np.asarray(inputs["fc2_b"], np.float32)])
            m["fc1_b"] = np.asarray(inputs["fc1_b"], np.float32)
        in_maps.append(m)
    return in_maps, ln_identity, zero_bias


def _assemble(results):
    out = np.empty((B, V, L, C), np.float32)
    for c in range(NCORES):
        b, g = divmod(c, G)
        oc = np.asarray(results[c]["out"])
        out[b, g] = oc[0:L]
        out[b, 7 - g] = oc[L:2 * L]
    return out


def kernel(**inputs):
    in_maps, ln_identity, zero_bias = _prep_inputs(inputs)
    nc = _get_nc(ln_identity, zero_bias)
    res = run_bass_kernel_spmd(nc, in_maps, core_ids=list(range(NCORES)))
    return _assemble(res.results)


# revision 29
# speedup vs baseline: 162.4995x; 1.0496x over previous
"""Trainium2 Bass kernel for nn_DecoderBlockBVL (B=2,V=8,L=256,C=768,H=12).

Sharding: 8 cores; core c handles batch b=c//4 and owns view pair
(vA, vB) = (g, 7-g) with g=c%4 (512 tokens). Phase 1 (per-view
self-attn) and the phase-2 k/v/q projections run only on owned views,
processing both views as one 512-token tile set. Phase-2 k and v are
exchanged in fp8 via two back-to-back AllGathers over the 4-core
batch group (K first, then V, so phase-2b scores overlap the V
gather); the gathered tensors are read back rank-major with a single
wide DMA fired from gpsimd the moment each collective lands.

The program is identical on every core (SPMD): phase-2b runs a
uniform 16 canonical key-blocks with both owned query views merged
into one 512-column moving operand, and the block-causal mask is
applied through a per-core data input `mbias` — an additive bias of
-30000 on the exp activation for out-of-prefix key chunks, which
zeroes those probabilities exactly. Canonical key block kb lives at a
fixed rank-major position (rank r holds views r and 7-r), so the
kb -> column mapping is the same Python constant on every core.

Attention is transpose-free: scores are computed keys-major (K-block
stationary, Q moving), exp'd in place (psum -> sbuf bf16), then used
directly as the moving operand of AV (V-slice stationary) and of a
ones-matmul that produces the softmax denominators replicated across
output partitions; normalization is one reciprocal + one tensor_mul
per head-pair. Head pairs pack the PE array: head 2i uses contract
rows / output cols 0-63, head 2i+1 uses 64-127. The head-pair loop is
software-pipelined: scores/exp of pair hp interleave with AV/sums of
pair hp-1 so scalar-exp latency never idles the PE.
"""

import numpy as np
import ml_dtypes
from contextlib import ExitStack

import concourse.bass as bass
import concourse.bacc as bacc
import concourse.mybir as mybir
import concourse.tile as tile
from concourse.bass_utils import run_bass_kernel_spmd
from concourse.masks import make_identity

dt = mybir.dt
F32 = dt.float32
BF16 = dt.bfloat16
FP8 = dt.float8e4
AF = mybir.ActivationFunctionType
ALU = mybir.AluOpType

B, V, L, C, H = 2, 8, 256, 768, 12
HD = C // H          # 64
S = V * L            # 2048
HID = 3072
NCORES = 8
G = 4                # cores per batch
MYQ = 2 * L          # 512 tokens per core (2 views)
SCALE = HD ** -0.5
CK = C // 128        # 6
HK = HID // 128      # 24
NHALF = ((0, 384), (384, 384))
NKB = 16             # uniform key-block count in phase 2b
MASKB = -30000.0     # exp bias for masked key chunks -> exp == 0.0

KZ = C * MYQ         # per-rank k zone [C, 512] in agin, elems
VZ = MYQ * C         # per-rank v zone [512, C]


def _kr(v):
    """allowed key prefix length for query view v (block-causal mask)"""
    return 512 if v < 2 else 256 * (v + 1)


def _kcol(kb):
    """canonical key block -> column offset in rank-major k2 [*, *, 2048]"""
    w, jj = kb // 2, kb % 2
    base = w * 512 if w < 4 else (7 - w) * 512 + 256
    return base + jj * 128


def _vblk(kb):
    """canonical key block -> token-block index in rank-major v2 [*, 16, C]"""
    w, jj = kb // 2, kb % 2
    return (w * 4 + jj) if w < 4 else ((7 - w) * 4 + 2 + jj)


def _build(ln_identity: bool, zero_bias: bool, sim_gelu: bool = False):
    nc = bacc.Bacc(num_devices=NCORES)

    xb = nc.declare_dram_parameter("xb", [MYQ, C], F32, isOutput=False)
    mbias = nc.declare_dram_parameter("mbias", [2, 16], F32, isOutput=False)
    wqkv = nc.declare_dram_parameter("wqkv_t", [C, 3 * C], BF16, isOutput=False)
    wproj = nc.declare_dram_parameter("wproj_t", [C, C], BF16, isOutput=False)
    wq = nc.declare_dram_parameter("wq_t", [C, C], BF16, isOutput=False)
    wk = nc.declare_dram_parameter("wk_t", [C, C], BF16, isOutput=False)
    wv = nc.declare_dram_parameter("wv_t", [C, C], BF16, isOutput=False)
    wcp = nc.declare_dram_parameter("wcproj_t", [C, C], BF16, isOutput=False)
    wf1 = nc.declare_dram_parameter("wfc1_t", [C, HID], BF16, isOutput=False)
    wf2 = nc.declare_dram_parameter("wfc2_t", [HID, C], BF16, isOutput=False)
    out = nc.declare_dram_parameter("out", [MYQ, C], F32, isOutput=True)

    lng = lnb = bias = f1b = None
    if not ln_identity:
        lng = nc.declare_dram_parameter("ln_g", [3, C], F32, isOutput=False)
        lnb = nc.declare_dram_parameter("ln_b", [3, C], F32, isOutput=False)
    if not zero_bias:
        bias = nc.declare_dram_parameter("bias3", [3, C], F32, isOutput=False)
        f1b = nc.declare_dram_parameter("fc1_b", [HID], F32, isOutput=False)

    rg = [[0, 1, 2, 3], [4, 5, 6, 7]]

    with tile.TileContext(nc) as tc, \
         tc.tile_pool(name="consts", bufs=1) as consts, \
         tc.tile_pool(name="dram", bufs=1, space="DRAM") as dram:
        identb = consts.tile([128, 128], BF16)
        make_identity(nc, identb)
        eps = consts.tile([128, 1], F32)
        nc.vector.memset(eps, 1e-5)
        ones64 = consts.tile([128, 64], BF16)
        nc.vector.memset(ones64, 1.0)

        def bcast_ap(handle):
            a = handle.ap()
            return bass.AP(tensor=a.tensor, offset=a.offset,
                           ap=[[0, 128]] + list(a.ap))

        mb_t = consts.tile([128, 2, 16], F32)
        nc.gpsimd.dma_start(out=mb_t[:], in_=bcast_ap(mbias))

        gbt = bbt = bias_bc = f1b_t = None
        if not ln_identity:
            gbt = consts.tile([128, 3, C], F32)
            bbt = consts.tile([128, 3, C], F32)
            for t, src in ((gbt, lng), (bbt, lnb)):
                nc.gpsimd.dma_start(out=t[:], in_=bcast_ap(src))
        if not zero_bias:
            bias_bc = consts.tile([128, 3, C], F32)
            nc.gpsimd.dma_start(out=bias_bc[:], in_=bcast_ap(bias))
            f1b_t = consts.tile([128, HK], F32)
            nc.gpsimd.dma_start(out=f1b_t[:], in_=f1b.rearrange("(a p) -> p a", p=128))

        agk = dram.tile([KZ], FP8)
        agv = dram.tile([VZ], FP8)
        agok = dram.tile([G * KZ], FP8)
        agov = dram.tile([G * VZ], FP8)

        def ln(pool, x_ap, h_ap, which):
            """layernorm over free dim C; x_ap/h_ap [128, C]"""
            st = pool.tile([128, 3, 6], F32, tag="ln_st")
            for sg in range(3):
                nc.vector.bn_stats(out=st[:, sg, :],
                                   in_=x_ap[:, sg * 256:(sg + 1) * 256])
            mv = pool.tile([128, 2], F32, tag="ln_mv")
            nc.vector.bn_aggr(out=mv[:], in_=st[:])
            nm = pool.tile([128, 2], F32, tag="ln_nm")  # [neg-mean, rstd]
            nc.vector.tensor_scalar_mul(nm[:, 0:1], mv[:, 0:1], -1.0)
            nc.scalar.activation(nm[:, 1:2], mv[:, 1:2], AF.Sqrt, bias=eps[:])
            nc.vector.reciprocal(nm[:, 1:2], nm[:, 1:2])
            nc.vector.tensor_scalar(h_ap, x_ap, nm[:, 0:1], nm[:, 1:2],
                                    ALU.add, ALU.mult)
            if not ln_identity:
                nc.vector.tensor_mul(h_ap, h_ap, gbt[:, which, :])
                nc.vector.tensor_add(h_ap, h_ap, bbt[:, which, :])

        def transpose_cols(psp, src, dst, j, n):
            """n [128,128] bf16 blocks src(mc) -> dst[:, j, :] ([128, n*128])"""
            ps = psp.tile([128, n * 128], BF16, tag="scb")
            for mc in range(n):
                nc.tensor.matmul(ps[:, mc * 128:(mc + 1) * 128], src(mc),
                                 identb[:], is_transpose=True)
            nc.any.tensor_copy(dst[:, j, :], ps[:])

        # ============ long-lived activation tiles ============
        with tc.tile_pool(name="res", bufs=1) as res, \
             tc.tile_pool(name="p2q", bufs=1) as p2q:
            x1own = res.tile([128, 4, C], F32)
            x2 = res.tile([128, 4, C], F32)
            q2T = p2q.tile([128, CK, MYQ], BF16)

            # ============ phase 1 + 2a (both owned views at once) ============
            with tc.tile_pool(name="p1w", bufs=1) as p1w, \
                 tc.tile_pool(name="p1b", bufs=1) as p1b, \
                 tc.tile_pool(name="p1s", bufs=4) as p1s, \
                 tc.tile_pool(name="ps_sc", bufs=1, space="PSUM") as ps_sc, \
                 tc.tile_pool(name="ps_mb", bufs=1, space="PSUM") as ps_mb, \
                 tc.tile_pool(name="ps_s", bufs=1, space="PSUM") as ps_s, \
                 tc.tile_pool(name="ps_o", bufs=1, space="PSUM") as ps_o:

                wqkv_s = p1w.tile([128, CK, 3 * C], BF16)
                wproj_s = p1w.tile([128, CK, C], BF16)
                wk_s = p1w.tile([128, CK, C], BF16)
                wv_s = p1w.tile([128, CK, C], BF16)
                wq_s = p1w.tile([128, CK, C], BF16)
                for kc in range(CK):
                    nc.sync.dma_start(out=wqkv_s[:, kc, :],
                                      in_=wqkv[kc * 128:(kc + 1) * 128, :])
                for wt, ws in ((wproj, wproj_s), (wk, wk_s), (wv, wv_s),
                               (wq, wq_s)):
                    for kc in range(CK):
                        nc.sync.dma_start(out=ws[:, kc, :],
                                          in_=wt[kc * 128:(kc + 1) * 128, :])

                # ---- phase 1: per-view self-attention, both views ----
                xv = p1b.tile([128, 4, C], F32)
                for mc in range(4):
                    nc.scalar.dma_start(
                        out=xv[:, mc, :],
                        in_=xb[mc * 128:(mc + 1) * 128, :])
                h1 = p1b.tile([128, 4, C], BF16)
                for mc in range(4):
                    ln(p1s, xv[:, mc, :], h1[:, mc, :], 0)
                h1T = p1b.tile([128, CK, MYQ], BF16)
                for j in range(CK):
                    transpose_cols(ps_sc,
                                   lambda mc: h1[:, mc, j * 128:(j + 1) * 128],
                                   h1T, j, 4)
                # q,k feature-major [1536, 512]
                qkT = p1b.tile([128, 12, MYQ], BF16)
                for mo in range(12):
                    ps = ps_sc.tile([128, MYQ], F32, tag="sc")
                    for kc in range(CK):
                        nc.tensor.matmul(ps[:],
                                         wqkv_s[:, kc, mo * 128:(mo + 1) * 128],
                                         h1T[:, kc, :],
                                         start=kc == 0, stop=kc == CK - 1)
                    nc.any.tensor_copy(qkT[:, mo, :], ps[:])
                # v token-major bf16 [512, 768]
                v1 = p1b.tile([128, 4, C], BF16)
                for mt in range(4):
                    pss = [ps_mb.tile([128, 384], F32, tag="mb",
                                      name=f"mbh{i}") for i in range(2)]
                    for kc in range(CK):
                        for i, (no, nn_) in enumerate(NHALF):
                            nc.tensor.matmul(pss[i][:],
                                             h1T[:, kc, mt * 128:(mt + 1) * 128],
                                             wqkv_s[:, kc, 2 * C + no:2 * C + no + nn_],
                                             start=kc == 0, stop=kc == CK - 1)
                    for i, (no, nn_) in enumerate(NHALF):
                        nc.any.tensor_copy(v1[:, mt, no:no + nn_], pss[i][:])

                # per-view attention (keys block-diagonal by view);
                # head-pair loop software-pipelined
                o1T = p1b.tile([128, CK, MYQ], BF16)
                prev = None
                for hp in range(7):
                    cur = None
                    if hp < 6:
                        eAB = []
                        for vi in range(2):
                            sA = ps_s.tile([128, 2, 256], F32, tag="sA")
                            sB = ps_s.tile([128, 2, 256], F32, tag="sB")
                            for j in range(2):
                                co = vi * 256
                                nc.tensor.matmul(
                                    sA[:, j, :],
                                    qkT[0:64, 6 + hp, co + j * 128:co + (j + 1) * 128],
                                    qkT[0:64, hp, co:co + 256],
                                    start=True, stop=True)
                                nc.tensor.matmul(
                                    sB[:, j, :],
                                    qkT[64:128, 6 + hp, co + j * 128:co + (j + 1) * 128],
                                    qkT[64:128, hp, co:co + 256],
                                    start=True, stop=True)
                            eA = p1s.tile([128, 2, 256], BF16, tag="eA1")
                            eB = p1s.tile([128, 2, 256], BF16, tag="eB1")
                            nc.scalar.activation(eA[:], sA[:], AF.Exp, scale=SCALE)
                            nc.scalar.activation(eB[:], sB[:], AF.Exp, scale=SCALE)
                            eAB.append((eA, eB))
                        cur = (hp, eAB)
                    if prev is not None:
                        php, peAB = prev
                        hA, hB = 2 * php, 2 * php + 1
                        o_ps = ps_o.tile([128, MYQ], F32, tag="o_ps")
                        s_ps = ps_o.tile([128, MYQ], F32, tag="s_ps")
                        for vi in range(2):
                            peA, peB = peAB[vi]
                            co = vi * 256
                            for kb in range(2):
                                st, sp = (kb == 0), (kb == 1)
                                nc.tensor.matmul(o_ps[0:64, co:co + 256],
                                                 v1[:, vi * 2 + kb, hA * 64:hA * 64 + 64],
                                                 peA[:, kb, :], start=st, stop=sp,
                                                 tile_position=(0, 0),
                                                 skip_group_check=True)
                                nc.tensor.matmul(o_ps[64:128, co:co + 256],
                                                 v1[:, vi * 2 + kb, hB * 64:hB * 64 + 64],
                                                 peB[:, kb, :], start=st, stop=sp,
                                                 tile_position=(0, 64),
                                                 skip_group_check=True)
                                nc.tensor.matmul(s_ps[0:64, co:co + 256], ones64[:],
                                                 peA[:, kb, :], start=st, stop=sp,
                                                 tile_position=(0, 0),
                                                 skip_group_check=True)
                                nc.tensor.matmul(s_ps[64:128, co:co + 256], ones64[:],
                                                 peB[:, kb, :], start=st, stop=sp,
                                                 tile_position=(0, 64),
                                                 skip_group_check=True)
                        rinv = p1s.tile([128, MYQ], F32, tag="rinv1")
                        nc.vector.reciprocal(rinv[:], s_ps[:])
                        nc.vector.tensor_mul(o1T[:, php, :], o_ps[:], rinv[:])
                    prev = cur

                # proj + residual -> x1own (token-major, stays in SBUF)
                for mt in range(4):
                    pss = [ps_mb.tile([128, 384], F32, tag="mb",
                                      name=f"mbh{i}") for i in range(2)]
                    for kc in range(CK):
                        for i, (no, nn_) in enumerate(NHALF):
                            nc.tensor.matmul(pss[i][:],
                                             o1T[:, kc, mt * 128:(mt + 1) * 128],
                                             wproj_s[:, kc, no:no + nn_],
                                             start=kc == 0, stop=kc == CK - 1)
                    for i, (no, nn_) in enumerate(NHALF):
                        nc.vector.tensor_add(x1own[:, mt, no:no + nn_],
                                             pss[i][:], xv[:, mt, no:no + nn_])
                    if not zero_bias:
                        nc.vector.tensor_add(x1own[:, mt, :], x1own[:, mt, :],
                                             bias_bc[:, 0, :])

                # ---- phase 2a: ln2 + k/v/q projections, both views ----
                h2 = p1b.tile([128, 4, C], BF16)
                for mc in range(4):
                    ln(p1s, x1own[:, mc, :], h2[:, mc, :], 1)
                h2T = p1b.tile([128, CK, MYQ], BF16)
                for j in range(CK):
                    transpose_cols(ps_sc,
                                   lambda mc: h2[:, mc, j * 128:(j + 1) * 128],
                                   h2T, j, 4)
                kown = p1b.tile([128, CK, MYQ], FP8)
                for mo in range(CK):
                    ps = ps_sc.tile([128, MYQ], F32, tag="sc")
                    for kc in range(CK):
                        nc.tensor.matmul(ps[:],
                                         wk_s[:, kc, mo * 128:(mo + 1) * 128],
                                         h2T[:, kc, :],
                                         start=kc == 0, stop=kc == CK - 1)
                    nc.any.tensor_copy(kown[:, mo, :], ps[:])
                nc.sync.dma_start(
                    out=bass.AP(tensor=agk.tensor, offset=agk.offset,
                                ap=[[MYQ, 128], [128 * MYQ, CK], [1, MYQ]]),
                    in_=kown[:])
                nc.gpsimd.collective_compute(
                    "AllGather", ALU.bypass, replica_groups=rg,
                    ins=[agk[:].opt()], outs=[agok[:].opt()])

                vown = p1b.tile([128, 4, C], FP8)
                for mt in range(4):
                    pss = [ps_mb.tile([128, 384], F32, tag="mb",
                                      name=f"mbh{i}") for i in range(2)]
                    for kc in range(CK):
                        for i, (no, nn_) in enumerate(NHALF):
                            nc.tensor.matmul(pss[i][:],
                                             h2T[:, kc, mt * 128:(mt + 1) * 128],
                                             wv_s[:, kc, no:no + nn_],
                                             start=kc == 0, stop=kc == CK - 1)
                    for i, (no, nn_) in enumerate(NHALF):
                        nc.any.tensor_copy(vown[:, mt, no:no + nn_], pss[i][:])
                nc.sync.dma_start(
                    out=bass.AP(tensor=agv.tensor, offset=agv.offset,
                                ap=[[C, 128], [128 * C, 4], [1, C]]),
                    in_=vown[:])
                nc.gpsimd.collective_compute(
                    "AllGather", ALU.bypass, replica_groups=rg,
                    ins=[agv[:].opt()], outs=[agov[:].opt()])

                # q projection (overlaps the collectives)
                for mo in range(CK):
                    ps = ps_sc.tile([128, MYQ], F32, tag="sc")
                    for kc in range(CK):
                        nc.tensor.matmul(ps[:],
                                         wq_s[:, kc, mo * 128:(mo + 1) * 128],
                                         h2T[:, kc, :],
                                         start=kc == 0, stop=kc == CK - 1)
                    nc.any.tensor_copy(q2T[:, mo, :], ps[:])

            # ====== k/v tiles (fp8, rank-major) + readback ======
            stack2 = ExitStack()
            p2kv = stack2.enter_context(tc.tile_pool(name="p2kv", bufs=1))
            p3w = stack2.enter_context(tc.tile_pool(name="p3w", bufs=1))
            k2 = p2kv.tile([128, CK, S], FP8)       # rank-major columns
            v2 = p2kv.tile([128, 16, C], FP8)       # rank-major token blocks
            o2T = p2kv.tile([128, CK, MYQ], BF16)
            # one wide rank-major readback per tensor, fired from gpsimd
            # as soon as the corresponding collective completes
            for r in range(G):
                nc.gpsimd.dma_start(
                    out=k2[:, :, r * MYQ:(r + 1) * MYQ],
                    in_=bass.AP(tensor=agok.tensor,
                                offset=agok.offset + r * KZ,
                                ap=[[MYQ, 128], [128 * MYQ, CK], [1, MYQ]]))
            for r in range(G):
                nc.gpsimd.dma_start(
                    out=v2[:, r * 4:(r + 1) * 4, :],
                    in_=bass.AP(tensor=agov.tensor,
                                offset=agov.offset + r * VZ,
                                ap=[[C, 128], [128 * C, 4], [1, C]]))

            # ====== phase 2b: block-causal cross-view attention ======
            # Query columns 0:256 = vA, 256:512 = vB. Canonical key block
            # kb covers view kb//2; kb < 8 (views 0-3) feeds both query
            # views, kb >= 8 only vB. K/V columns are addressed rank-major
            # via _kcol/_vblk (a fixed mapping, identical on every core).
            with tc.tile_pool(name="p2bs", bufs=2) as p2bs, \
                 tc.tile_pool(name="ps2b_s", bufs=1, space="PSUM") as ps2b_s, \
                 tc.tile_pool(name="ps2b_o", bufs=2, space="PSUM") as ps2b_o:
                wcp_s = p3w.tile([128, CK, C], BF16)
                wf1_s = p3w.tile([128, CK, HID], BF16)
                prev = None
                for hp in range(7):
                    cur = None
                    if hp < 6:
                        eA = p2bs.tile([128, NKB, MYQ], BF16, tag="eA")
                        eB = p2bs.tile([128, NKB, MYQ], BF16, tag="eB")
                        cur = (hp, eA, eB)
                    if prev is not None:
                        o_ps = ps2b_o.tile([128, MYQ], F32, tag="o_ps")
                        s_ps = ps2b_o.tile([128, MYQ], F32, tag="s_ps")
                    for kbp in range(NKB // 2):
                        if hp < 6:
                            sA = ps2b_s.tile([128, 2, MYQ], F32, tag="sA")
                            sB = ps2b_s.tile([128, 2, MYQ], F32, tag="sB")
                            for j in range(2):
                                kb = 2 * kbp + j
                                kc0 = _kcol(kb)
                                co = 0 if kb < 8 else 256
                                nc.tensor.matmul(
                                    sA[:, j, co:MYQ],
                                    k2[0:64, hp, kc0:kc0 + 128],
                                    q2T[0:64, hp, co:MYQ],
                                    start=True, stop=True)
                                nc.tensor.matmul(
                                    sB[:, j, co:MYQ],
                                    k2[64:128, hp, kc0:kc0 + 128],
                                    q2T[64:128, hp, co:MYQ],
                                    start=True, stop=True)
                            for s_, e_ in ((sA, eA), (sB, eB)):
                                eo = e_[:, 2 * kbp:2 * kbp + 2, :]
                                if kbp < 2:
                                    # kb 0-3: no view is ever masked here
                                    nc.scalar.activation(eo[:, :, :], s_[:],
                                                         AF.Exp, scale=SCALE)
                                elif kbp < 4:
                                    # kb 4-7: vA may be masked; vB never
                                    nc.scalar.activation(
                                        eo[:, :, 0:256], s_[:, :, 0:256],
                                        AF.Exp, scale=SCALE,
                                        bias=mb_t[:, 0, kbp:kbp + 1])
                                    nc.scalar.activation(
                                        eo[:, :, 256:MYQ], s_[:, :, 256:MYQ],
                                        AF.Exp, scale=SCALE)
                                else:
                                    # kb 8-15: vB only, may be masked
                                    nc.scalar.activation(
                                        eo[:, :, 256:MYQ], s_[:, :, 256:MYQ],
                                        AF.Exp, scale=SCALE,
                                        bias=mb_t[:, 1, kbp:kbp + 1])
                        if prev is not None:
                            php, peA, peB = prev
                            hA, hB = 2 * php, 2 * php + 1
                            for j in range(2):
                                kb = 2 * kbp + j
                                vb = _vblk(kb)
                                co = 0 if kb < 8 else 256
                                st, sp = (kb == 0), (kb == NKB - 1)
                                nc.tensor.matmul(o_ps[0:64, co:MYQ],
                                                 v2[:, vb, hA * 64:hA * 64 + 64],
                                                 peA[:, kb, co:MYQ],
                                                 start=st, stop=sp,
                                                 tile_position=(0, 0),
                                                 skip_group_check=True)
                                nc.tensor.matmul(o_ps[64:128, co:MYQ],
                                                 v2[:, vb, hB * 64:hB * 64 + 64],
                                                 peB[:, kb, co:MYQ],
                                                 start=st, stop=sp,
                                                 tile_position=(0, 64),
                                                 skip_group_check=True)
                                nc.tensor.matmul(s_ps[0:64, co:MYQ], ones64[:],
                                                 peA[:, kb, co:MYQ],
                                                 start=st, stop=sp,
                                                 tile_position=(0, 0),
                                                 skip_group_check=True)
                                nc.tensor.matmul(s_ps[64:128, co:MYQ], ones64[:],
                                                 peB[:, kb, co:MYQ],
                                                 start=st, stop=sp,
                                                 tile_position=(0, 64),
                                                 skip_group_check=True)
                    if prev is not None:
                        php = prev[0]
                        rinv = p2bs.tile([128, MYQ], F32, tag="rinv")
                        nc.vector.reciprocal(rinv[:], s_ps[:])
                        nc.vector.tensor_mul(o2T[:, php, :], o_ps[:], rinv[:])
                    prev = cur
                    # stagger the phase-2c/3 weight loads into the scalar
                    # queue mid-phase so they don't contend with the
                    # collectives or the k/v readbacks
                    if hp == 1:
                        for kc in range(CK):
                            nc.scalar.dma_start(out=wcp_s[:, kc, :],
                                                in_=wcp[kc * 128:(kc + 1) * 128, :])
                    elif hp == 2:
                        for kc in range(CK):
                            nc.scalar.dma_start(out=wf1_s[:, kc, :],
                                                in_=wf1[kc * 128:(kc + 1) * 128, :])
            # ====== phase 2c: cproj + residual (+ fc2 prefetch) ======
            p3w2 = stack2.enter_context(tc.tile_pool(name="p3w2", bufs=1))
            with tc.tile_pool(name="ps2c", bufs=3, space="PSUM") as ps2c:
                wf2_s = p3w2.tile([128, HK, C], BF16)
                for kc in range(HK):
                    nc.scalar.dma_start(out=wf2_s[:, kc, :],
                                        in_=wf2[kc * 128:(kc + 1) * 128, :])
                for mt in range(4):
                    pss = [ps2c.tile([128, 384], F32, tag="mb",
                                     name=f"mbh{i}") for i in range(2)]
                    for kc in range(CK):
                        for i, (no, nn_) in enumerate(NHALF):
                            nc.tensor.matmul(pss[i][:],
                                             o2T[:, kc, mt * 128:(mt + 1) * 128],
                                             wcp_s[:, kc, no:no + nn_],
                                             start=kc == 0, stop=kc == CK - 1)
                    for i, (no, nn_) in enumerate(NHALF):
                        nc.vector.tensor_add(x2[:, mt, no:no + nn_], pss[i][:],
                                             x1own[:, mt, no:no + nn_])
                    if not zero_bias:
                        nc.vector.tensor_add(x2[:, mt, :], x2[:, mt, :],
                                             bias_bc[:, 1, :])

            # ====== phase 3: MLP ======
            with tc.tile_pool(name="p3one", bufs=1) as p3one, \
                 tc.tile_pool(name="p3s", bufs=2) as p3s, \
                 tc.tile_pool(name="ps3_sc", bufs=2, space="PSUM") as ps3_sc, \
                 tc.tile_pool(name="ps3_mb", bufs=3, space="PSUM") as ps3_mb:
                h3 = p3one.tile([128, 4, C], BF16)
                for mt in range(4):
                    ln(p3s, x2[:, mt, :], h3[:, mt, :], 2)
                h3T = p3one.tile([128, CK, MYQ], BF16)
                for j in range(CK):
                    transpose_cols(ps3_sc,
                                   lambda mc: h3[:, mc, j * 128:(j + 1) * 128],
                                   h3T, j, 4)
                g1T = p3one.tile([128, HK, MYQ], BF16)
                for mo in range(HK):
                    ps = ps3_sc.tile([128, MYQ], F32, tag="sc")
                    for kc in range(CK):
                        nc.tensor.matmul(ps[:], wf1_s[:, kc, mo * 128:(mo + 1) * 128],
                                         h3T[:, kc, :], start=kc == 0, stop=kc == CK - 1)
                    if sim_gelu:
                        # tanh-approx gelu from sim-supported ops (sim only)
                        xg = p3s.tile([128, MYQ], F32, tag="xg")
                        if zero_bias:
                            nc.any.tensor_copy(xg[:], ps[:])
                        else:
                            nc.scalar.activation(xg[:], ps[:], AF.Identity,
                                                 bias=f1b_t[:, mo:mo + 1])
                        x2g = p3s.tile([128, MYQ], F32, tag="x2g")
                        nc.scalar.activation(x2g[:], xg[:], AF.Square)
                        nc.vector.tensor_scalar(x2g[:], x2g[:], 0.0356774081,
                                                0.7978845608, ALU.mult, ALU.add)
                        nc.vector.tensor_mul(x2g[:], x2g[:], xg[:])
                        nc.scalar.activation(x2g[:], x2g[:], AF.Tanh)
                        nc.vector.tensor_mul(x2g[:], x2g[:], xg[:])
                        nc.vector.tensor_add(x2g[:], x2g[:], xg[:])
                        nc.vector.tensor_scalar_mul(x2g[:], x2g[:], 0.5)
                        nc.any.tensor_copy(g1T[:, mo, :], x2g[:])
                    elif zero_bias:
                        nc.scalar.activation(g1T[:, mo, :], ps[:], AF.Gelu)
                    else:
                        nc.scalar.activation(g1T[:, mo, :], ps[:], AF.Gelu,
                                             bias=f1b_t[:, mo:mo + 1])
                for mt in range(4):
                    pss = [ps3_mb.tile([128, 384], F32, tag="mb",
                                       name=f"mbh{i}") for i in range(2)]
                    for kc in range(HK):
                        for i, (no, nn_) in enumerate(NHALF):
                            nc.tensor.matmul(pss[i][:],
                                             g1T[:, kc, mt * 128:(mt + 1) * 128],
                                             wf2_s[:, kc, no:no + nn_],
                                             start=kc == 0, stop=kc == HK - 1)
                    yo = p3s.tile([128, C], F32, tag="yo")
                    for i, (no, nn_) in enumerate(NHALF):
                        nc.vector.tensor_add(yo[:, no:no + nn_], pss[i][:],
                                             x2[:, mt, no:no + nn_])
                    if not zero_bias:
                        nc.vector.tensor_add(yo[:], yo[:], bias_bc[:, 2, :])
                    nc.sync.dma_start(out=out[mt * 128:(mt + 1) * 128, :], in_=yo[:])
            stack2.close()

    nc.finalize()
    return nc


_CACHE = {}


def _get_nc(ln_identity, zero_bias, sim_gelu=False):
    key = (ln_identity, zero_bias, sim_gelu)
    if key not in _CACHE:
        _CACHE[key] = _build(ln_identity, zero_bias, sim_gelu)
    return _CACHE[key]


def _prep_inputs(inputs):
    x = np.asarray(inputs["x"], np.float32)          # [B, V, L, C]
    ln_identity = all(np.all(np.asarray(inputs[f"ln{i}_g"]) == 1.0)
                      and np.all(np.asarray(inputs[f"ln{i}_b"]) == 0.0)
                      for i in (1, 2, 3))
    zero_bias = all(np.all(np.asarray(inputs[k]) == 0.0)
                    for k in ("attn_proj_b", "cproj_b", "fc1_b", "fc2_b"))

    tr = lambda k: np.ascontiguousarray(
        np.asarray(inputs[k], np.float32).T).astype(ml_dtypes.bfloat16)
    wqkv_t, wproj_t = tr("qkv_w"), tr("attn_proj_w")
    wq_t, wk_t, wv_t, wcp_t = tr("q_w"), tr("k_w"), tr("v_w"), tr("cproj_w")
    wf1_t = tr("fc1_w")
    wf2_t = tr("fc2_w")

    in_maps = []
    for c in range(NCORES):
        b, g = divmod(c, G)
        xbp = np.concatenate([x[b, g], x[b, 7 - g]], axis=0)  # [512, C]
        mb = np.zeros((2, 16), np.float32)
        for vi, v in enumerate((g, 7 - g)):
            mb[vi, _kr(v) // 256:] = MASKB
        m = {"xb": np.ascontiguousarray(xbp), "mbias": mb,
             "wqkv_t": wqkv_t, "wproj_t": wproj_t, "wq_t": wq_t,
             "wk_t": wk_t, "wv_t": wv_t, "wcproj_t": wcp_t,
             "wfc1_t": wf1_t, "wfc2_t": wf2_t}
        if not ln_identity:
            m["ln_g"] = np.stack([np.asarray(inputs[f"ln{i}_g"], np.float32)
                                  for i in (1, 2, 3)])
            m["ln_b"] = np.stack([np.asarray(inputs[f"ln{i}_b"], np.float32)
                                  for i in (1, 2, 3)])
        if not zero_bias:
            m["bias3"] = np.stack([np.asarray(inputs["attn_proj_b"], np.float32),
                                   np.asarray(inputs["cproj_b"], np.float32),
                                   np.asarray(inputs["fc2_b"], np.float32)])
            m["fc1_b"] = np.asarray(inputs["fc1_b"], np.float32)
        in_maps.append(m)
    return in_maps, ln_identity, zero_bias


def _assemble(results):
    out = np.empty((B, V, L, C), np.float32)
    for c in range(NCORES):
        b, g = divmod(c, G)
        oc = np.asarray(results[c]["out"])
        out[b, g] = oc[0:L]
        out[b, 7 - g] = oc[L:2 * L]
    return out


def kernel(**inputs):
    in_maps, ln_identity, zero_bias = _prep_inputs(inputs)
    nc = _get_nc(ln_identity, zero_bias)
    res = run_bass_kernel_spmd(nc, in_maps, core_ids=list(range(NCORES)))
    return _assemble(res.results)
